# revision 4
# baseline (speedup 1.0000x reference)
"""Trainium2 Bass kernel for a 2-layer LSTM motion-prior sampler (DBSE).

Computes, per time step t (B=2048, T=64, Z=64, H=512):
    h1,c1 = LSTMCell(x_t, h1, c1; W_ih1, W_hh1, b1)     x_t = z_post[:, t-1] (0 at t=0)
    h2,c2 = LSTMCell(h1, h2, c2; W_ih2, W_hh2, b2)
    m_t   = h2 @ W_mean.T + b_mean
    lv_t  = h2 @ W_logvar.T + b_logvar
    z_t   = m_t + eps_t * exp(0.5 * lv_t)
Returns (z_means, z_logvars, z_out), each [B, T, Z] float32.

Sharding: data-parallel over batch across 8 NeuronCores (256 rows each),
weights replicated, recurrence local per core, no collectives.

Device layout: everything feature-major [feature, batch] so LSTM matmul
outputs land directly in recurrence layout with zero transposes.
Matmul inputs are bf16 (fp32 PSUM accumulation); c-state and all
elementwise math stay fp32. Biases are folded into the matmuls (ones-row
trick for layer 1, K=1 bias matmuls for layer 2 / heads). The exp() of
the reparameterization runs in a final phase so the ScalarEngine keeps a
single sigmoid/tanh activation-table set during the whole recurrence.
"""

import os
import sys

for _p in ("/opt/trn_rl_repo", "/root/.axon_site/_ro/trn_rl_repo"):
    if os.path.isdir(_p) and _p not in sys.path:
        sys.path.insert(0, _p)

import numpy as np
import ml_dtypes

B = 2048
T = int(os.environ.get("KERNEL_T", "64"))
Z = 64
H = 512
G = 4 * H           # 2048 gate units per layer
NCORES = 8
BL = B // NCORES    # 256 batch rows per core
KT = H // 128       # 4 contraction k-tiles of 128
TC = 8              # time-chunk for the final reparameterization phase
BF = ml_dtypes.bfloat16

_CACHE = {}


def _build_bass():
    import concourse.tile as tile
    from concourse import bacc, mybir

    f32 = mybir.dt.float32
    bf = mybir.dt.bfloat16
    AF = mybir.ActivationFunctionType

    nc = bacc.Bacc(None, target_bir_lowering=False)

    # ---------------- DRAM I/O (per core) ----------------
    xT = nc.dram_tensor("xT", [T, Z + 1, BL], bf, kind="ExternalInput")
    epsT = nc.dram_tensor("epsT", [T, Z, BL], f32, kind="ExternalInput")
    w1x_d = nc.dram_tensor("w1x", [Z + 1, G], bf, kind="ExternalInput")
    whh1_d = nc.dram_tensor("whh1", [H, G], bf, kind="ExternalInput")
    wih2_d = nc.dram_tensor("wih2", [H, G], bf, kind="ExternalInput")
    whh2_d = nc.dram_tensor("whh2", [H, G], bf, kind="ExternalInput")
    whd_d = nc.dram_tensor("whd", [H, 2 * Z], bf, kind="ExternalInput")
    b2_d = nc.dram_tensor("b2", [1, G], bf, kind="ExternalInput")
    bhd_d = nc.dram_tensor("bhd", [1, 2 * Z], bf, kind="ExternalInput")
    zm_d = nc.dram_tensor("zm", [T, Z, BL], f32, kind="ExternalOutput")
    zlv_d = nc.dram_tensor("zlv", [T, Z, BL], f32, kind="ExternalOutput")
    zz_d = nc.dram_tensor("zz", [T, Z, BL], f32, kind="ExternalOutput")

    # psum chunk position -> weight-column base; gates ordered [i,f,o | g]
    # in PSUM so one sigmoid covers [0:1536] and one tanh covers [1536:2048].
    # PyTorch weight rows are [i, f, g, o] blocks of H.
    GBASE = [0 * H, 1 * H, 3 * H, 2 * H]  # i, f, o, g

    with tile.TileContext(nc) as tc:
        from contextlib import ExitStack

        with ExitStack() as ctx:
            state = ctx.enter_context(tc.tile_pool(name="state", bufs=1))
            dram = ctx.enter_context(
                tc.tile_pool(name="dram", bufs=1, space="DRAM")
            )

            # ---------------- persistent SBUF ----------------
            w1x = state.tile([Z + 1, G], bf)
            nc.sync.dma_start(w1x[:], w1x_d[:])
            whh1 = state.tile([128, KT, G], bf)
            nc.sync.dma_start(whh1[:], whh1_d.rearrange("(k p) g -> p k g", p=128))
            wih2 = state.tile([128, KT, G], bf)
            nc.sync.dma_start(wih2[:], wih2_d.rearrange("(k p) g -> p k g", p=128))
            whh2 = state.tile([128, KT, G], bf)
            nc.sync.dma_start(whh2[:], whh2_d.rearrange("(k p) g -> p k g", p=128))
            whd = state.tile([128, KT, 2 * Z], bf)
            nc.sync.dma_start(whd[:], whd_d.rearrange("(k p) g -> p k g", p=128))
            b2 = state.tile([1, G], bf)
            nc.sync.dma_start(b2[:], b2_d[:])
            bhd = state.tile([1, 2 * Z], bf)
            nc.sync.dma_start(bhd[:], bhd_d[:])
            xall = state.tile([Z + 1, T, BL], bf)
            nc.sync.dma_start(xall[:], xT.rearrange("t p b -> p t b"))

            ones = state.tile([1, BL], bf)
            nc.gpsimd.memset(ones[:], 1.0)

            # double-buffered h (bf16, matmul input); single-buffer c (fp32)
            h1b0 = state.tile([128, KT * BL], bf)
            h1b1 = state.tile([128, KT * BL], bf)
            h2b0 = state.tile([128, KT * BL], bf)
            h2b1 = state.tile([128, KT * BL], bf)
            c1 = state.tile([128, KT * BL], f32)
            c2 = state.tile([128, KT * BL], f32)
            for buf in (h1b0, h1b1, h2b0, h2b1):
                nc.gpsimd.memset(buf[:], 0.0)
            nc.gpsimd.memset(c1[:], 0.0)
            nc.gpsimd.memset(c2[:], 0.0)
            h1 = (h1b0, h1b1)
            h2 = (h2b0, h2b1)

            # m|lv per-step scratch in DRAM, read back by the final phase
            mlv_dram = dram.tile([T, 128, BL], f32)

            with (
                tc.tile_pool(name="psum", bufs=2, space="PSUM") as psum,
                tc.tile_pool(name="work", bufs=2) as work,
            ):

                def l1_mms(t, hf, h_cur):
                    """Layer-1 gate matmuls for half hf -> psum tile."""
                    pg = psum.tile([128, G], f32, tag="g", name=f"pg1_{t}_{hf}")
                    for pos in range(4):
                        for j in range(2):
                            off = pos * 512 + j * 256
                            col = GBASE[pos] + (2 * hf + j) * 128
                            nc.tensor.matmul(
                                pg[:, off : off + 256],
                                w1x[:, col : col + 128],
                                xall[:, t, :],
                                start=True,
                                stop=False,
                            )
                            for kc in range(KT):
                                nc.tensor.matmul(
                                    pg[:, off : off + 256],
                                    whh1[:, kc, col : col + 128],
                                    h_cur[:, kc * 256 : (kc + 1) * 256],
                                    start=False,
                                    stop=(kc == KT - 1),
                                )
                    return pg

                def l2_mms(t, hf, h1_new, h2_cur):
                    """Layer-2 gate matmuls for half hf -> psum tile."""
                    pg = psum.tile([128, G], f32, tag="g", name=f"pg2_{t}_{hf}")
                    for pos in range(4):
                        for j in range(2):
                            off = pos * 512 + j * 256
                            col = GBASE[pos] + (2 * hf + j) * 128
                            nc.tensor.matmul(
                                pg[:, off : off + 256],
                                b2[:, col : col + 128],
                                ones[:],
                                start=True,
                                stop=False,
                            )
                            for kc in range(KT):
                                nc.tensor.matmul(
                                    pg[:, off : off + 256],
                                    whh2[:, kc, col : col + 128],
                                    h2_cur[:, kc * 256 : (kc + 1) * 256],
                                    start=False,
                                    stop=False,
                                )
                            for kc in range(KT):
                                nc.tensor.matmul(
                                    pg[:, off : off + 256],
                                    wih2[:, kc, col : col + 128],
                                    h1_new[:, kc * 256 : (kc + 1) * 256],
                                    start=False,
                                    stop=(kc == KT - 1),
                                )
                    return pg

                def eltwise(t, lname, pgs, c, h_new):
                    """LSTM cell elementwise for both halves of a layer.

                    ACT order sigmoid(h0), tanh_g(h0), sigmoid(h1),
                    tanh_g(h1), tanh_c(h0), tanh_c(h1) keeps the scalar
                    engine busy while the DVE computes the c-updates.
                    """
                    ifo = [None, None]
                    gg = [None, None]
                    th = [None, None]
                    for hf in (0, 1):
                        ifo[hf] = work.tile(
                            [128, 1536], f32, tag="ifo", name=f"ifo_{lname}_{t}_{hf}"
                        )
                        nc.scalar.activation(
                            ifo[hf][:], pgs[hf][:, 0:1536], AF.Sigmoid
                        )
                        gg[hf] = work.tile(
                            [128, 512], f32, tag="gg", name=f"gg_{lname}_{t}_{hf}"
                        )
                        nc.scalar.activation(
                            gg[hf][:], pgs[hf][:, 1536:2048], AF.Tanh
                        )
                    for hf in (0, 1):
                        cs = c[:, hf * 512 : (hf + 1) * 512]
                        t1 = work.tile([128, 512], f32, tag="t1", name=f"t1_{lname}_{t}_{hf}")
                        nc.vector.tensor_mul(t1[:], ifo[hf][:, 512:1024], cs)
                        t2 = work.tile([128, 512], f32, tag="t2", name=f"t2_{lname}_{t}_{hf}")
                        nc.vector.tensor_mul(t2[:], ifo[hf][:, 0:512], gg[hf][:])
                        nc.vector.tensor_add(cs, t1[:], t2[:])
                    for hf in (0, 1):
                        th[hf] = work.tile(
                            [128, 512], f32, tag="th", name=f"th_{lname}_{t}_{hf}"
                        )
                        nc.scalar.activation(
                            th[hf][:], c[:, hf * 512 : (hf + 1) * 512], AF.Tanh
                        )
                    for hf in (0, 1):
                        nc.vector.tensor_mul(
                            h_new[:, hf * 512 : (hf + 1) * 512],
                            ifo[hf][:, 1024:1536],
                            th[hf][:],
                        )

                def heads(t, h2_new):
                    """mean|logvar head for step t -> mlv_dram[t]."""
                    ph = psum.tile([128, G], f32, tag="g", name=f"ph_{t}")
                    nc.tensor.matmul(
                        ph[:, 0:BL], bhd[:], ones[:], start=True, stop=False
                    )
                    for kc in range(KT):
                        nc.tensor.matmul(
                            ph[:, 0:BL],
                            whd[:, kc, :],
                            h2_new[:, kc * 256 : (kc + 1) * 256],
                            start=False,
                            stop=(kc == KT - 1),
                        )
                    mlv = work.tile([128, BL], f32, tag="mlv", name=f"mlv_{t}")
                    nc.vector.tensor_copy(mlv[:], ph[:, 0:BL])
                    nc.sync.dma_start(mlv_dram[t], mlv[:])

                # ---------------- recurrence ----------------
                for t in range(T):
                    cur, nxt = t % 2, (t + 1) % 2
                    pg1 = [l1_mms(t, hf, h1[cur]) for hf in (0, 1)]
                    eltwise(t, "l1", pg1, c1, h1[nxt])
                    if t > 0:
                        heads(t - 1, h2[cur])
                    pg2 = [l2_mms(t, hf, h1[nxt], h2[cur]) for hf in (0, 1)]
                    eltwise(t, "l2", pg2, c2, h2[nxt])
                heads(T - 1, h2[T % 2])

            # ---------------- final phase: z = m + eps * exp(lv/2) ----------
            TC_ = min(TC, T)
            with tc.tile_pool(name="fin", bufs=2) as fin:
                for t0 in range(0, T, TC_):
                    m_sb = fin.tile([Z, TC_, BL], f32, tag="m", name=f"m_{t0}")
                    nc.sync.dma_start(
                        m_sb[:],
                        mlv_dram[t0 : t0 + TC_, 0:Z, :].rearrange("t p b -> p t b"),
                    )
                    lv_sb = fin.tile([Z, TC_, BL], f32, tag="lv", name=f"lv_{t0}")
                    nc.sync.dma_start(
                        lv_sb[:],
                        mlv_dram[t0 : t0 + TC_, Z : 2 * Z, :].rearrange(
                            "t p b -> p t b"
                        ),
                    )
                    eps_sb = fin.tile([Z, TC_, BL], f32, tag="eps", name=f"eps_{t0}")
                    nc.sync.dma_start(
                        eps_sb[:], epsT[t0 : t0 + TC_].rearrange("t p b -> p t b")
                    )
                    e_sb = fin.tile([Z, TC_, BL], f32, tag="e", name=f"e_{t0}")
                    nc.scalar.activation(e_sb[:], lv_sb[:], AF.Exp, scale=0.5)
                    z_sb = fin.tile([Z, TC_, BL], f32, tag="z", name=f"z_{t0}")
                    nc.vector.tensor_mul(z_sb[:], e_sb[:], eps_sb[:])
                    nc.vector.tensor_add(z_sb[:], z_sb[:], m_sb[:])
                    nc.sync.dma_start(
                        zm_d[t0 : t0 + TC_].rearrange("t p b -> p t b"), m_sb[:]
                    )
                    nc.sync.dma_start(
                        zlv_d[t0 : t0 + TC_].rearrange("t p b -> p t b"), lv_sb[:]
                    )
                    nc.sync.dma_start(
                        zz_d[t0 : t0 + TC_].rearrange("t p b -> p t b"), z_sb[:]
                    )

    nc.compile()
    return nc


def _get_nc():
    if "nc" not in _CACHE:
        _CACHE["nc"] = _build_bass()
    return _CACHE["nc"]


def kernel(z_post, eps, W_ih1, W_hh1, b_ih1, b_hh1, W_ih2, W_hh2, b_ih2, b_hh2,
           W_mean, b_mean, W_logvar, b_logvar):
    z_post = np.asarray(z_post, np.float32)
    eps = np.asarray(eps, np.float32)
    W_ih1 = np.asarray(W_ih1, np.float32)
    W_hh1 = np.asarray(W_hh1, np.float32)
    W_ih2 = np.asarray(W_ih2, np.float32)
    W_hh2 = np.asarray(W_hh2, np.float32)
    W_mean = np.asarray(W_mean, np.float32)
    W_logvar = np.asarray(W_logvar, np.float32)
    b1 = (np.asarray(b_ih1, np.float32) + np.asarray(b_hh1, np.float32))
    b2 = (np.asarray(b_ih2, np.float32) + np.asarray(b_hh2, np.float32))
    b_mean = np.asarray(b_mean, np.float32)
    b_logvar = np.asarray(b_logvar, np.float32)

    # shared (replicated) weight tensors, pre-transposed for the PE array
    w1x = np.ascontiguousarray(
        np.concatenate([W_ih1.T, b1[None, :]], 0)
    ).astype(BF)                                          # [Z+1, G]
    whh1 = np.ascontiguousarray(W_hh1.T).astype(BF)       # [H, G]
    wih2 = np.ascontiguousarray(W_ih2.T).astype(BF)
    whh2 = np.ascontiguousarray(W_hh2.T).astype(BF)
    whd = np.ascontiguousarray(
        np.concatenate([W_mean, W_logvar], 0).T
    ).astype(BF)                                          # [H, 2Z]
    b2v = b2[None, :].astype(BF)
    bhd = np.concatenate([b_mean, b_logvar])[None, :].astype(BF)

    # x_t is the previous frame's z_post
    x_seq = np.concatenate(
        [np.zeros_like(z_post[:, :1]), z_post[:, : T - 1]], 1
    )                                                     # [B, T, Z]

    in_maps = []
    for ci in range(NCORES):
        sl = slice(ci * BL, (ci + 1) * BL)
        xc = np.ascontiguousarray(x_seq[sl].transpose(1, 2, 0))     # [T, Z, BL]
        xc = np.concatenate(
            [xc, np.ones((T, 1, BL), np.float32)], 1
        ).astype(BF)                                                # [T, Z+1, BL]
        epsc = np.ascontiguousarray(eps[sl, :T].transpose(1, 2, 0)) # [T, Z, BL]
        in_maps.append(
            {
                "xT": xc,
                "epsT": epsc,
                "w1x": w1x,
                "whh1": whh1,
                "wih2": wih2,
                "whh2": whh2,
                "whd": whd,
                "b2": b2v,
                "bhd": bhd,
            }
        )

    from concourse.bass_utils import run_bass_kernel_spmd

    nc = _get_nc()
    trace = os.environ.get("KERNEL_PROFILE", "") == "1"
    res = run_bass_kernel_spmd(
        nc, in_maps, core_ids=list(range(NCORES)), trace=trace
    )
    if trace:
        _CACHE["exec_time_ns"] = res.exec_time_ns

    outs = []
    for name in ("zm", "zlv", "zz"):
        parts = [
            res.results[ci][name].transpose(2, 0, 1)  # [T,Z,BL] -> [BL,T,Z]
            for ci in range(NCORES)
        ]
        outs.append(np.ascontiguousarray(np.concatenate(parts, 0), np.float32))
    return tuple(outs)


# revision 6
# speedup vs baseline: 1.1910x; 1.1910x over previous
"""Trainium2 Bass kernel for a 2-layer LSTM motion-prior sampler (DBSE).

Computes, per time step t (B=2048, T=64, Z=64, H=512):
    h1,c1 = LSTMCell(x_t, h1, c1; W_ih1, W_hh1, b1)     x_t = z_post[:, t-1] (0 at t=0)
    h2,c2 = LSTMCell(h1, h2, c2; W_ih2, W_hh2, b2)
    m_t   = h2 @ W_mean.T + b_mean
    lv_t  = h2 @ W_logvar.T + b_logvar
    z_t   = m_t + eps_t * exp(0.5 * lv_t)
Returns (z_means, z_logvars, z_out), each [B, T, Z] float32.

Sharding: data-parallel over batch across 8 NeuronCores (256 rows each),
weights replicated, recurrence local per core, no collectives.

Device layout: everything feature-major [feature, batch] so LSTM matmul
outputs land directly in recurrence layout with zero transposes.
Matmul inputs are bf16 (fp32 PSUM accumulation); c-state and all
elementwise math stay fp32. Biases are folded into the matmuls (ones-row
trick for layer 1, K=1 bias matmuls for layer 2 / heads).

exp(0.5*lv) is computed in-loop as s/(1-s) with s = sigmoid(0.5*lv) so
the ScalarEngine keeps a single sigmoid/tanh activation-table set for
the whole kernel (an exp table switch costs ~2.7us and would serialize).

Layer-2 matmuls are emitted source-major (bias, then h2-dependent, then
h1-dependent k-tiles) within each PSUM-bank-safe half so the PE absorbs
the layer-1 elementwise latency instead of stalling. PSUM accumulation
groups sharing a bank must not interleave their start-matmuls (the
first matmul of a group clears has_written for the whole bank), hence
the j=0 block fully precedes the j=1 block.
"""

import os
import sys

for _p in ("/opt/trn_rl_repo", "/root/.axon_site/_ro/trn_rl_repo"):
    if os.path.isdir(_p) and _p not in sys.path:
        sys.path.insert(0, _p)

import numpy as np
import ml_dtypes

B = 2048
T = int(os.environ.get("KERNEL_T", "64"))
Z = 64
H = 512
G = 4 * H           # 2048 gate units per layer
NCORES = 8
BL = B // NCORES    # 256 batch rows per core
KT = H // 128       # 4 contraction k-tiles of 128
BF = ml_dtypes.bfloat16

_CACHE = {}


def _build_bass():
    import concourse.tile as tile
    from concourse import bacc, mybir

    f32 = mybir.dt.float32
    bf = mybir.dt.bfloat16
    AF = mybir.ActivationFunctionType
    OP = mybir.AluOpType

    nc = bacc.Bacc(None, target_bir_lowering=False)

    # ---------------- DRAM I/O (per core) ----------------
    xT = nc.dram_tensor("xT", [T, Z + 1, BL], bf, kind="ExternalInput")
    epsT = nc.dram_tensor("epsT", [T, Z, BL], f32, kind="ExternalInput")
    w1x_d = nc.dram_tensor("w1x", [Z + 1, G], bf, kind="ExternalInput")
    whh1_d = nc.dram_tensor("whh1", [H, G], bf, kind="ExternalInput")
    wih2_d = nc.dram_tensor("wih2", [H, G], bf, kind="ExternalInput")
    whh2_d = nc.dram_tensor("whh2", [H, G], bf, kind="ExternalInput")
    whd_d = nc.dram_tensor("whd", [H, 2 * Z], bf, kind="ExternalInput")
    b2_d = nc.dram_tensor("b2", [1, G], bf, kind="ExternalInput")
    bhd_d = nc.dram_tensor("bhd", [1, 2 * Z], bf, kind="ExternalInput")
    zm_d = nc.dram_tensor("zm", [T, Z, BL], f32, kind="ExternalOutput")
    zlv_d = nc.dram_tensor("zlv", [T, Z, BL], f32, kind="ExternalOutput")
    zz_d = nc.dram_tensor("zz", [T, Z, BL], f32, kind="ExternalOutput")

    # psum chunk position -> weight-column base; gates ordered [i,f,o | g]
    # in PSUM so one sigmoid covers [0:1536] and one tanh covers [1536:2048].
    # PyTorch weight rows are [i, f, g, o] blocks of H.
    GBASE = [0 * H, 1 * H, 3 * H, 2 * H]  # i, f, o, g

    with tile.TileContext(nc) as tc:
        from contextlib import ExitStack

        with ExitStack() as ctx:
            state = ctx.enter_context(tc.tile_pool(name="state", bufs=1))

            # ---------------- persistent SBUF ----------------
            # order matters: the t=0 layer-1 matmuls only need w1x/x/whh1,
            # so those transfers go first and the loop starts sooner.
            w1x = state.tile([Z + 1, G], bf)
            nc.sync.dma_start(w1x[:], w1x_d[:])
            xall = state.tile([Z + 1, T, BL], bf)
            nc.sync.dma_start(xall[:], xT.rearrange("t p b -> p t b"))
            whh1 = state.tile([128, KT, G], bf)
            nc.sync.dma_start(whh1[:], whh1_d.rearrange("(k p) g -> p k g", p=128))
            whh2 = state.tile([128, KT, G], bf)
            nc.sync.dma_start(whh2[:], whh2_d.rearrange("(k p) g -> p k g", p=128))
            wih2 = state.tile([128, KT, G], bf)
            nc.sync.dma_start(wih2[:], wih2_d.rearrange("(k p) g -> p k g", p=128))
            whd = state.tile([128, KT, 2 * Z], bf)
            nc.sync.dma_start(whd[:], whd_d.rearrange("(k p) g -> p k g", p=128))
            b2 = state.tile([1, G], bf)
            nc.sync.dma_start(b2[:], b2_d[:])
            bhd = state.tile([1, 2 * Z], bf)
            nc.sync.dma_start(bhd[:], bhd_d[:])

            ones = state.tile([1, BL], bf)
            nc.gpsimd.memset(ones[:], 1.0)

            # double-buffered h (bf16, matmul input); single-buffer c (fp32)
            h1b0 = state.tile([128, KT * BL], bf)
            h1b1 = state.tile([128, KT * BL], bf)
            h2b0 = state.tile([128, KT * BL], bf)
            h2b1 = state.tile([128, KT * BL], bf)
            c1 = state.tile([128, KT * BL], f32)
            c2 = state.tile([128, KT * BL], f32)
            for buf in (h1b0, h1b1, h2b0, h2b1):
                nc.gpsimd.memset(buf[:], 0.0)
            nc.gpsimd.memset(c1[:], 0.0)
            nc.gpsimd.memset(c2[:], 0.0)
            h1 = (h1b0, h1b1)
            h2 = (h2b0, h2b1)

            with (
                tc.tile_pool(name="psum", bufs=2, space="PSUM") as psum,
                tc.tile_pool(name="work", bufs=2) as work,
                tc.tile_pool(name="epsp", bufs=4) as epsp,
            ):

                def l1_mms(t, hf, h_cur):
                    """Layer-1 gate matmuls for half hf -> psum tile."""
                    pg = psum.tile([128, G], f32, tag="g", name=f"pg1_{t}_{hf}")
                    for j in (0, 1):
                        cols = [GBASE[pos] + (2 * hf + j) * 128 for pos in range(4)]
                        offs = [pos * 512 + j * 256 for pos in range(4)]
                        for pos in range(4):
                            nc.tensor.matmul(
                                pg[:, offs[pos] : offs[pos] + 256],
                                w1x[:, cols[pos] : cols[pos] + 128],
                                xall[:, t, :],
                                start=True,
                                stop=False,
                            )
                        for kc in range(KT):
                            for pos in range(4):
                                nc.tensor.matmul(
                                    pg[:, offs[pos] : offs[pos] + 256],
                                    whh1[:, kc, cols[pos] : cols[pos] + 128],
                                    h_cur[:, kc * 256 : (kc + 1) * 256],
                                    start=False,
                                    stop=(kc == KT - 1),
                                )
                    return pg

                def l2_mms(t, hf, h1_new, h2_cur):
                    """Layer-2 gate matmuls, source-major: the h1-dependent
                    matmuls come last so the PE keeps streaming while the
                    layer-1 elementwise chain finishes."""
                    pg = psum.tile([128, G], f32, tag="g", name=f"pg2_{t}_{hf}")
                    for j in (0, 1):
                        cols = [GBASE[pos] + (2 * hf + j) * 128 for pos in range(4)]
                        offs = [pos * 512 + j * 256 for pos in range(4)]
                        for pos in range(4):
                            nc.tensor.matmul(
                                pg[:, offs[pos] : offs[pos] + 256],
                                b2[:, cols[pos] : cols[pos] + 128],
                                ones[:],
                                start=True,
                                stop=False,
                            )
                        for kc in range(KT):
                            for pos in range(4):
                                nc.tensor.matmul(
                                    pg[:, offs[pos] : offs[pos] + 256],
                                    whh2[:, kc, cols[pos] : cols[pos] + 128],
                                    h2_cur[:, kc * 256 : (kc + 1) * 256],
                                    start=False,
                                    stop=False,
                                )
                        for kc in range(KT):
                            for pos in range(4):
                                nc.tensor.matmul(
                                    pg[:, offs[pos] : offs[pos] + 256],
                                    wih2[:, kc, cols[pos] : cols[pos] + 128],
                                    h1_new[:, kc * 256 : (kc + 1) * 256],
                                    start=False,
                                    stop=(kc == KT - 1),
                                )
                    return pg

                def eltwise(t, lname, pgs, c, h_new):
                    """LSTM cell elementwise for both halves of a layer.

                    ACT runs both halves' sigmoid/tanh_g before the
                    tanh_c's so it stays busy while the DVE does the
                    c-updates; tanh_c output is bf16 (h is stored bf16
                    anyway) which also speeds the final multiply."""
                    ifo = [None, None]
                    gg = [None, None]
                    th = [None, None]
                    for hf in (0, 1):
                        ifo[hf] = work.tile(
                            [128, 1536], f32, tag="ifo", name=f"ifo_{lname}_{t}_{hf}"
                        )
                        nc.scalar.activation(ifo[hf][:], pgs[hf][:, 0:1536], AF.Sigmoid)
                        gg[hf] = work.tile(
                            [128, 512], f32, tag="gg", name=f"gg_{lname}_{t}_{hf}"
                        )
                        nc.scalar.activation(gg[hf][:], pgs[hf][:, 1536:2048], AF.Tanh)
                    for hf in (0, 1):
                        cs = c[:, hf * 512 : (hf + 1) * 512]
                        t1 = work.tile([128, 512], f32, tag="t1", name=f"t1_{lname}_{t}_{hf}")
                        nc.vector.tensor_mul(t1[:], ifo[hf][:, 512:1024], cs)
                        t2 = work.tile([128, 512], f32, tag="t2", name=f"t2_{lname}_{t}_{hf}")
                        nc.vector.tensor_mul(t2[:], ifo[hf][:, 0:512], gg[hf][:])
                        nc.vector.tensor_add(cs, t1[:], t2[:])
                    for hf in (0, 1):
                        th[hf] = work.tile(
                            [128, 512], bf, tag="th", name=f"th_{lname}_{t}_{hf}"
                        )
                        nc.scalar.activation(
                            th[hf][:], c[:, hf * 512 : (hf + 1) * 512], AF.Tanh
                        )
                    for hf in (0, 1):
                        nc.vector.tensor_mul(
                            h_new[:, hf * 512 : (hf + 1) * 512],
                            ifo[hf][:, 1024:1536],
                            th[hf][:],
                        )

                def heads_mm(t, h2_new):
                    """mean|logvar head matmuls for step t -> SBUF copy."""
                    ph = psum.tile([128, G], f32, tag="g", name=f"ph_{t}")
                    nc.tensor.matmul(ph[:, 0:BL], bhd[:], ones[:], start=True, stop=False)
                    for kc in range(KT):
                        nc.tensor.matmul(
                            ph[:, 0:BL],
                            whd[:, kc, :],
                            h2_new[:, kc * 256 : (kc + 1) * 256],
                            start=False,
                            stop=(kc == KT - 1),
                        )
                    mlv = work.tile([128, BL], f32, tag="mlv", name=f"mlv_{t}")
                    nc.vector.tensor_copy(mlv[:], ph[:, 0:BL])
                    return mlv

                def heads_z(t, mlv):
                    """z_t = m + eps * exp(0.5*lv), with exp via the
                    sigmoid table set: exp(x) = s/(1-s), s = sigmoid(x/2).
                    lv lives on partitions 64..127; a small SBUF->SBUF DMA
                    aligns it with m on partitions 0..63."""
                    lvs = work.tile([Z, BL], f32, tag="lvs", name=f"lvs_{t}")
                    nc.sync.dma_start(lvs[:], mlv[Z : 2 * Z, :])
                    s = work.tile([Z, BL], f32, tag="s", name=f"s_{t}")
                    nc.scalar.activation(s[:], lvs[:], AF.Sigmoid, scale=0.5)
                    u = work.tile([Z, BL], f32, tag="u", name=f"u_{t}")
                    nc.vector.tensor_scalar(u[:], s[:], -1.0, 1.0, OP.mult, OP.add)
                    r = work.tile([Z, BL], f32, tag="r", name=f"r_{t}")
                    nc.vector.reciprocal(r[:], u[:])
                    e = work.tile([Z, BL], f32, tag="e", name=f"e_{t}")
                    nc.vector.tensor_mul(e[:], s[:], r[:])
                    epst = epsp.tile([Z, BL], f32, tag="eps", name=f"eps_{t}")
                    nc.sync.dma_start(epst[:], epsT[t])
                    zt = work.tile([Z, BL], f32, tag="zt", name=f"zt_{t}")
                    nc.vector.tensor_mul(zt[:], e[:], epst[:])
                    nc.vector.tensor_add(zt[:], zt[:], mlv[0:Z, :])
                    nc.sync.dma_start(zm_d[t], mlv[0:Z, :])
                    nc.sync.dma_start(zlv_d[t], lvs[:])
                    nc.sync.dma_start(zz_d[t], zt[:])

                # ---------------- recurrence ----------------
                # heads/z for step t-1 are threaded through step t so their
                # matmuls slot into the PE stream where inputs are ready.
                mlv_p = None
                for t in range(T):
                    cur, nxt = t % 2, (t + 1) % 2
                    pg1 = [l1_mms(t, hf, h1[cur]) for hf in (0, 1)]
                    eltwise(t, "l1", pg1, c1, h1[nxt])
                    if t > 0:
                        mlv_p = heads_mm(t - 1, h2[cur])
                    pg2 = [l2_mms(t, hf, h1[nxt], h2[cur]) for hf in (0, 1)]
                    eltwise(t, "l2", pg2, c2, h2[nxt])
                    if t > 0:
                        heads_z(t - 1, mlv_p)
                mlv_p = heads_mm(T - 1, h2[T % 2])
                heads_z(T - 1, mlv_p)

    nc.compile()
    return nc


def _get_nc():
    if "nc" not in _CACHE:
        _CACHE["nc"] = _build_bass()
    return _CACHE["nc"]


def kernel(z_post, eps, W_ih1, W_hh1, b_ih1, b_hh1, W_ih2, W_hh2, b_ih2, b_hh2,
           W_mean, b_mean, W_logvar, b_logvar):
    z_post = np.asarray(z_post, np.float32)
    eps = np.asarray(eps, np.float32)
    W_ih1 = np.asarray(W_ih1, np.float32)
    W_hh1 = np.asarray(W_hh1, np.float32)
    W_ih2 = np.asarray(W_ih2, np.float32)
    W_hh2 = np.asarray(W_hh2, np.float32)
    W_mean = np.asarray(W_mean, np.float32)
    W_logvar = np.asarray(W_logvar, np.float32)
    b1 = (np.asarray(b_ih1, np.float32) + np.asarray(b_hh1, np.float32))
    b2 = (np.asarray(b_ih2, np.float32) + np.asarray(b_hh2, np.float32))
    b_mean = np.asarray(b_mean, np.float32)
    b_logvar = np.asarray(b_logvar, np.float32)

    # shared (replicated) weight tensors, pre-transposed for the PE array
    w1x = np.ascontiguousarray(
        np.concatenate([W_ih1.T, b1[None, :]], 0)
    ).astype(BF)                                          # [Z+1, G]
    whh1 = np.ascontiguousarray(W_hh1.T).astype(BF)       # [H, G]
    wih2 = np.ascontiguousarray(W_ih2.T).astype(BF)
    whh2 = np.ascontiguousarray(W_hh2.T).astype(BF)
    whd = np.ascontiguousarray(
        np.concatenate([W_mean, W_logvar], 0).T
    ).astype(BF)                                          # [H, 2Z]
    b2v = b2[None, :].astype(BF)
    bhd = np.concatenate([b_mean, b_logvar])[None, :].astype(BF)

    # x_t is the previous frame's z_post
    x_seq = np.concatenate(
        [np.zeros_like(z_post[:, :1]), z_post[:, : T - 1]], 1
    )                                                     # [B, T, Z]

    in_maps = []
    for ci in range(NCORES):
        sl = slice(ci * BL, (ci + 1) * BL)
        xc = np.ascontiguousarray(x_seq[sl].transpose(1, 2, 0))     # [T, Z, BL]
        xc = np.concatenate(
            [xc, np.ones((T, 1, BL), np.float32)], 1
        ).astype(BF)                                                # [T, Z+1, BL]
        epsc = np.ascontiguousarray(eps[sl, :T].transpose(1, 2, 0)) # [T, Z, BL]
        in_maps.append(
            {
                "xT": xc,
                "epsT": epsc,
                "w1x": w1x,
                "whh1": whh1,
                "wih2": wih2,
                "whh2": whh2,
                "whd": whd,
                "b2": b2v,
                "bhd": bhd,
            }
        )

    from concourse.bass_utils import run_bass_kernel_spmd

    nc = _get_nc()
    trace = os.environ.get("KERNEL_PROFILE", "") == "1"
    res = run_bass_kernel_spmd(
        nc, in_maps, core_ids=list(range(NCORES)), trace=trace
    )
    if trace:
        _CACHE["exec_time_ns"] = res.exec_time_ns

    outs = []
    for name in ("zm", "zlv", "zz"):
        parts = [
            res.results[ci][name].transpose(2, 0, 1)  # [T,Z,BL] -> [BL,T,Z]
            for ci in range(NCORES)
        ]
        outs.append(np.ascontiguousarray(np.concatenate(parts, 0), np.float32))
    return tuple(outs)


# revision 10
# speedup vs baseline: 1.2484x; 1.0482x over previous
"""Trainium2 Bass kernel for a 2-layer LSTM motion-prior sampler (DBSE).

Computes, per time step t (B=2048, T=64, Z=64, H=512):
    h1,c1 = LSTMCell(x_t, h1, c1; W_ih1, W_hh1, b1)     x_t = z_post[:, t-1] (0 at t=0)
    h2,c2 = LSTMCell(h1, h2, c2; W_ih2, W_hh2, b2)
    m_t   = h2 @ W_mean.T + b_mean
    lv_t  = h2 @ W_logvar.T + b_logvar
    z_t   = m_t + eps_t * exp(0.5 * lv_t)
Returns (z_means, z_logvars, z_out), each [B, T, Z] float32.

Sharding: data-parallel over batch across 8 NeuronCores (256 rows each),
weights replicated, recurrence local per core, no collectives.

Device layout: everything feature-major [feature, batch] so LSTM matmul
outputs land directly in recurrence layout with zero transposes.
Matmul inputs are bf16 (fp32 PSUM accumulation); c-state and all
elementwise math stay fp32. Biases are folded into the matmuls (ones-row
trick for layer 1, K=1 bias matmuls for layer 2 / heads).

exp(0.5*lv) is computed in-loop as s/(1-s) with s = sigmoid(0.5*lv) so
the ScalarEngine keeps a single sigmoid/tanh activation-table set for
the whole kernel (an exp table switch costs ~2.7us and would serialize).

Layer-2 matmuls are emitted source-major (bias, then h2-dependent, then
h1-dependent k-tiles) within each PSUM-bank-safe half so the PE absorbs
the layer-1 elementwise latency instead of stalling. PSUM accumulation
groups sharing a bank must not interleave their start-matmuls (the
first matmul of a group clears has_written for the whole bank), hence
the j=0 block fully precedes the j=1 block.
"""

import os
import sys

for _p in ("/opt/trn_rl_repo", "/root/.axon_site/_ro/trn_rl_repo"):
    if os.path.isdir(_p) and _p not in sys.path:
        sys.path.insert(0, _p)

import numpy as np
import ml_dtypes

B = 2048
T = int(os.environ.get("KERNEL_T", "64"))
Z = 64
H = 512
G = 4 * H           # 2048 gate units per layer
NCORES = 8
BL = B // NCORES    # 256 batch rows per core
KT = H // 128       # 4 contraction k-tiles of 128
BF = ml_dtypes.bfloat16

_CACHE = {}


def _build_bass():
    import concourse.tile as tile
    from concourse import bacc, mybir

    f32 = mybir.dt.float32
    bf = mybir.dt.bfloat16
    AF = mybir.ActivationFunctionType
    OP = mybir.AluOpType

    nc = bacc.Bacc(None, target_bir_lowering=False)

    # ---------------- DRAM I/O (per core) ----------------
    xT = nc.dram_tensor("xT", [T, Z + 1, BL], bf, kind="ExternalInput")
    epsT = nc.dram_tensor("epsT", [T, Z, BL], f32, kind="ExternalInput")
    w1x_d = nc.dram_tensor("w1x", [Z + 1, G], bf, kind="ExternalInput")
    whh1_d = nc.dram_tensor("whh1", [H, G], bf, kind="ExternalInput")
    wih2_d = nc.dram_tensor("wih2", [H, G], bf, kind="ExternalInput")
    whh2_d = nc.dram_tensor("whh2", [H, G], bf, kind="ExternalInput")
    whd_d = nc.dram_tensor("whd", [H, 2 * Z], bf, kind="ExternalInput")
    b2_d = nc.dram_tensor("b2", [97, G], bf, kind="ExternalInput")
    bhd_d = nc.dram_tensor("bhd", [1, 2 * Z], bf, kind="ExternalInput")
    zm_d = nc.dram_tensor("zm", [T, Z, BL], f32, kind="ExternalOutput")
    zlv_d = nc.dram_tensor("zlv", [T, Z, BL], f32, kind="ExternalOutput")
    zz_d = nc.dram_tensor("zz", [T, Z, BL], f32, kind="ExternalOutput")

    # psum chunk position -> weight-column base; gates ordered [i,f,o | g]
    # in PSUM so one sigmoid covers [0:1536] and one tanh covers [1536:2048].
    # PyTorch weight rows are [i, f, g, o] blocks of H.
    GBASE = [0 * H, 1 * H, 3 * H, 2 * H]  # i, f, o, g

    with tile.TileContext(nc) as tc:
        from contextlib import ExitStack

        with ExitStack() as ctx:
            state = ctx.enter_context(tc.tile_pool(name="state", bufs=1))

            # ---------------- persistent SBUF ----------------
            # order matters: the t=0 layer-1 matmuls only need w1x/x/whh1,
            # so those transfers go first and the loop starts sooner.
            w1x = state.tile([Z + 1, G], bf)
            nc.sync.dma_start(w1x[:], w1x_d[:])
            xall = state.tile([Z + 1, T, BL], bf)
            nc.sync.dma_start(xall[:], xT.rearrange("t p b -> p t b"))
            whh1 = state.tile([128, KT, G], bf)
            for kc in range(KT):
                nc.sync.dma_start(
                    whh1[:, kc, :], whh1_d[kc * 128 : (kc + 1) * 128, :]
                )
            whh2 = state.tile([128, KT, G], bf)
            for kc in range(KT):
                nc.sync.dma_start(
                    whh2[:, kc, :], whh2_d[kc * 128 : (kc + 1) * 128, :]
                )
            wih2 = state.tile([128, KT, G], bf)
            for kc in range(KT):
                nc.sync.dma_start(
                    wih2[:, kc, :], wih2_d[kc * 128 : (kc + 1) * 128, :]
                )
            whd = state.tile([128, KT, 2 * Z], bf)
            nc.sync.dma_start(whd[:], whd_d.rearrange("(k p) g -> p k g", p=128))
            b2 = state.tile([97, G], bf)
            nc.sync.dma_start(b2[:], b2_d[:])
            bhd = state.tile([1, 2 * Z], bf)
            nc.sync.dma_start(bhd[:], bhd_d[:])

            ones = state.tile([97, BL], bf)
            nc.gpsimd.memset(ones[:], 1.0)

            # double-buffered h (bf16, matmul input); single-buffer c (fp32)
            h1b0 = state.tile([128, KT * BL], bf)
            h1b1 = state.tile([128, KT * BL], bf)
            h2b0 = state.tile([128, KT * BL], bf)
            h2b1 = state.tile([128, KT * BL], bf)
            c1 = state.tile([128, KT * BL], f32)
            c2 = state.tile([128, KT * BL], f32)
            for buf in (h1b0, h1b1, h2b0, h2b1):
                nc.gpsimd.memset(buf[:], 0.0)
            nc.gpsimd.memset(c1[:], 0.0)
            nc.gpsimd.memset(c2[:], 0.0)
            h1 = (h1b0, h1b1)
            h2 = (h2b0, h2b1)

            with (
                tc.tile_pool(name="psum", bufs=2, space="PSUM") as psum,
                tc.tile_pool(name="work", bufs=2) as work,
                tc.tile_pool(name="epsp", bufs=4) as epsp,
            ):

                def l1_mms(t, hf, h_cur):
                    """Layer-1 gate matmuls for half hf -> psum tile."""
                    pg = psum.tile([128, G], f32, tag="g", name=f"pg1_{t}_{hf}")
                    for j in (0, 1):
                        cols = [GBASE[pos] + (2 * hf + j) * 128 for pos in range(4)]
                        offs = [pos * 512 + j * 256 for pos in range(4)]
                        for pos in range(4):
                            nc.tensor.matmul(
                                pg[:, offs[pos] : offs[pos] + 256],
                                w1x[:, cols[pos] : cols[pos] + 128],
                                xall[:, t, :],
                                start=True,
                                stop=False,
                            )
                        for kc in range(KT):
                            for pos in range(4):
                                nc.tensor.matmul(
                                    pg[:, offs[pos] : offs[pos] + 256],
                                    whh1[:, kc, cols[pos] : cols[pos] + 128],
                                    h_cur[:, kc * 256 : (kc + 1) * 256],
                                    start=False,
                                    stop=(kc == KT - 1),
                                )
                    return pg

                def l2_bias_h2(t, hf, j, pg, h2_cur):
                    """j-block prefix of layer-2 half hf: row-packed K=1
                    bias matmuls (4 concurrent row strips) + the
                    h2-dependent k-tiles. All inputs ready at step start."""
                    cols = [GBASE[pos] + (2 * hf + j) * 128 for pos in range(4)]
                    offs = [pos * 512 + j * 256 for pos in range(4)]
                    for pos in range(4):
                        r = 32 * pos
                        nc.tensor.matmul(
                            pg[:, offs[pos] : offs[pos] + 256],
                            b2[r : r + 1, cols[pos] : cols[pos] + 128],
                            ones[r : r + 1, :],
                            start=True,
                            stop=False,
                            tile_position=(r, 0),
                        )
                    for kc in range(KT):
                        for pos in range(4):
                            nc.tensor.matmul(
                                pg[:, offs[pos] : offs[pos] + 256],
                                whh2[:, kc, cols[pos] : cols[pos] + 128],
                                h2_cur[:, kc * 256 : (kc + 1) * 256],
                                start=False,
                                stop=False,
                            )

                def l2_h1(t, hf, j, pg, h1_new):
                    """j-block suffix of layer-2 half hf: the h1-dependent
                    k-tiles, emitted last so the PE absorbs the layer-1
                    elementwise latency."""
                    cols = [GBASE[pos] + (2 * hf + j) * 128 for pos in range(4)]
                    offs = [pos * 512 + j * 256 for pos in range(4)]
                    for kc in range(KT):
                        for pos in range(4):
                            nc.tensor.matmul(
                                pg[:, offs[pos] : offs[pos] + 256],
                                wih2[:, kc, cols[pos] : cols[pos] + 128],
                                h1_new[:, kc * 256 : (kc + 1) * 256],
                                start=False,
                                stop=(kc == KT - 1),
                            )

                def eltwise(t, lname, pgs, c, h_new):
                    """LSTM cell elementwise for both halves of a layer.

                    ACT runs both halves' sigmoid/tanh_g before the
                    tanh_c's so it stays busy while the DVE does the
                    c-updates; tanh_c output is bf16 (h is stored bf16
                    anyway) which also speeds the final multiply."""
                    ifo = [None, None]
                    gg = [None, None]
                    th = [None, None]
                    for hf in (0, 1):
                        ifo[hf] = work.tile(
                            [128, 1536], f32, tag="ifo", name=f"ifo_{lname}_{t}_{hf}"
                        )
                        nc.scalar.activation(ifo[hf][:], pgs[hf][:, 0:1536], AF.Sigmoid)
                        gg[hf] = work.tile(
                            [128, 512], f32, tag="gg", name=f"gg_{lname}_{t}_{hf}"
                        )
                        nc.scalar.activation(gg[hf][:], pgs[hf][:, 1536:2048], AF.Tanh)
                    for hf in (0, 1):
                        cs = c[:, hf * 512 : (hf + 1) * 512]
                        t1 = work.tile([128, 512], f32, tag="t1", name=f"t1_{lname}_{t}_{hf}")
                        nc.vector.tensor_mul(t1[:], ifo[hf][:, 512:1024], cs)
                        t2 = work.tile([128, 512], f32, tag="t2", name=f"t2_{lname}_{t}_{hf}")
                        nc.vector.tensor_mul(t2[:], ifo[hf][:, 0:512], gg[hf][:])
                        nc.vector.tensor_add(cs, t1[:], t2[:])
                    for hf in (0, 1):
                        th[hf] = work.tile(
                            [128, 512], bf, tag="th", name=f"th_{lname}_{t}_{hf}"
                        )
                        nc.scalar.activation(
                            th[hf][:], c[:, hf * 512 : (hf + 1) * 512], AF.Tanh
                        )
                    for hf in (0, 1):
                        nc.vector.tensor_mul(
                            h_new[:, hf * 512 : (hf + 1) * 512],
                            ifo[hf][:, 1024:1536],
                            th[hf][:],
                        )

                def heads_mm(t, h2_new):
                    """mean|logvar head matmuls for step t -> SBUF copy."""
                    ph = psum.tile([128, G], f32, tag="g", name=f"ph_{t}")
                    nc.tensor.matmul(
                        ph[:, 0:BL], bhd[:], ones[0:1, :], start=True, stop=False
                    )
                    for kc in range(KT):
                        nc.tensor.matmul(
                            ph[:, 0:BL],
                            whd[:, kc, :],
                            h2_new[:, kc * 256 : (kc + 1) * 256],
                            start=False,
                            stop=(kc == KT - 1),
                        )
                    mlv = work.tile([128, BL], f32, tag="mlv", name=f"mlv_{t}")
                    nc.vector.tensor_copy(mlv[:], ph[:, 0:BL])
                    return mlv

                def heads_z(t, mlv):
                    """z_t = m + eps * exp(0.5*lv), with exp via the
                    sigmoid table set: exp(x) = s/(1-s), s = sigmoid(x/2).
                    lv lives on partitions 64..127; a small SBUF->SBUF DMA
                    aligns it with m on partitions 0..63."""
                    lvs = work.tile([Z, BL], f32, tag="lvs", name=f"lvs_{t}")
                    nc.sync.dma_start(lvs[:], mlv[Z : 2 * Z, :])
                    s = work.tile([Z, BL], f32, tag="s", name=f"s_{t}")
                    nc.scalar.activation(s[:], lvs[:], AF.Sigmoid, scale=0.5)
                    u = work.tile([Z, BL], f32, tag="u", name=f"u_{t}")
                    nc.vector.tensor_scalar(u[:], s[:], -1.0, 1.0, OP.mult, OP.add)
                    r = work.tile([Z, BL], f32, tag="r", name=f"r_{t}")
                    nc.vector.reciprocal(r[:], u[:])
                    e = work.tile([Z, BL], f32, tag="e", name=f"e_{t}")
                    nc.vector.tensor_mul(e[:], s[:], r[:])
                    epst = epsp.tile([Z, BL], f32, tag="eps", name=f"eps_{t}")
                    nc.sync.dma_start(epst[:], epsT[t])
                    zt = work.tile([Z, BL], f32, tag="zt", name=f"zt_{t}")
                    nc.vector.tensor_mul(zt[:], e[:], epst[:])
                    nc.vector.tensor_add(zt[:], zt[:], mlv[0:Z, :])
                    nc.sync.dma_start(zm_d[t], mlv[0:Z, :])
                    nc.sync.dma_start(zlv_d[t], lvs[:])
                    nc.sync.dma_start(zz_d[t], zt[:])

                # ---------------- recurrence ----------------
                # heads/z for step t-1 run at the tail of step t where the
                # PE has slack; layer-2 j-blocks interleave across halves
                # so a long run of h2-dependent matmuls precedes the first
                # h1-dependent one (absorbing the layer-1 eltwise chain).
                for t in range(T):
                    cur, nxt = t % 2, (t + 1) % 2
                    pg1 = [l1_mms(t, hf, h1[cur]) for hf in (0, 1)]
                    eltwise(t, "l1", pg1, c1, h1[nxt])
                    pg2 = [
                        psum.tile([128, G], f32, tag="g", name=f"pg2_{t}_{hf}")
                        for hf in (0, 1)
                    ]
                    for j in (0, 1):
                        for hf in (0, 1):
                            l2_bias_h2(t, hf, j, pg2[hf], h2[cur])
                        for hf in (0, 1):
                            l2_h1(t, hf, j, pg2[hf], h1[nxt])
                    eltwise(t, "l2", pg2, c2, h2[nxt])
                    if t > 0:
                        heads_z(t - 1, heads_mm(t - 1, h2[cur]))
                heads_z(T - 1, heads_mm(T - 1, h2[T % 2]))

    nc.compile()
    return nc


def _get_nc():
    if "nc" not in _CACHE:
        _CACHE["nc"] = _build_bass()
    return _CACHE["nc"]


def kernel(z_post, eps, W_ih1, W_hh1, b_ih1, b_hh1, W_ih2, W_hh2, b_ih2, b_hh2,
           W_mean, b_mean, W_logvar, b_logvar):
    z_post = np.asarray(z_post, np.float32)
    eps = np.asarray(eps, np.float32)
    W_ih1 = np.asarray(W_ih1, np.float32)
    W_hh1 = np.asarray(W_hh1, np.float32)
    W_ih2 = np.asarray(W_ih2, np.float32)
    W_hh2 = np.asarray(W_hh2, np.float32)
    W_mean = np.asarray(W_mean, np.float32)
    W_logvar = np.asarray(W_logvar, np.float32)
    b1 = (np.asarray(b_ih1, np.float32) + np.asarray(b_hh1, np.float32))
    b2 = (np.asarray(b_ih2, np.float32) + np.asarray(b_hh2, np.float32))
    b_mean = np.asarray(b_mean, np.float32)
    b_logvar = np.asarray(b_logvar, np.float32)

    # shared (replicated) weight tensors, pre-transposed for the PE array
    w1x = np.ascontiguousarray(
        np.concatenate([W_ih1.T, b1[None, :]], 0)
    ).astype(BF)                                          # [Z+1, G]
    whh1 = np.ascontiguousarray(W_hh1.T).astype(BF)       # [H, G]
    wih2 = np.ascontiguousarray(W_ih2.T).astype(BF)
    whh2 = np.ascontiguousarray(W_hh2.T).astype(BF)
    whd = np.ascontiguousarray(
        np.concatenate([W_mean, W_logvar], 0).T
    ).astype(BF)                                          # [H, 2Z]
    # bias rows replicated at partitions 0/32/64/96 for row-packed K=1 matmuls
    b2v = np.zeros((97, G), np.float32)
    for r in (0, 32, 64, 96):
        b2v[r] = b2
    b2v = b2v.astype(BF)
    bhd = np.concatenate([b_mean, b_logvar])[None, :].astype(BF)

    # x_t is the previous frame's z_post
    x_seq = np.concatenate(
        [np.zeros_like(z_post[:, :1]), z_post[:, : T - 1]], 1
    )                                                     # [B, T, Z]

    in_maps = []
    for ci in range(NCORES):
        sl = slice(ci * BL, (ci + 1) * BL)
        xc = np.ascontiguousarray(x_seq[sl].transpose(1, 2, 0))     # [T, Z, BL]
        xc = np.concatenate(
            [xc, np.ones((T, 1, BL), np.float32)], 1
        ).astype(BF)                                                # [T, Z+1, BL]
        epsc = np.ascontiguousarray(eps[sl, :T].transpose(1, 2, 0)) # [T, Z, BL]
        in_maps.append(
            {
                "xT": xc,
                "epsT": epsc,
                "w1x": w1x,
                "whh1": whh1,
                "wih2": wih2,
                "whh2": whh2,
                "whd": whd,
                "b2": b2v,
                "bhd": bhd,
            }
        )

    from concourse.bass_utils import run_bass_kernel_spmd

    nc = _get_nc()
    trace = os.environ.get("KERNEL_PROFILE", "") == "1"
    res = run_bass_kernel_spmd(
        nc, in_maps, core_ids=list(range(NCORES)), trace=trace
    )
    if trace:
        _CACHE["exec_time_ns"] = res.exec_time_ns

    outs = []
    for name in ("zm", "zlv", "zz"):
        parts = [
            res.results[ci][name].transpose(2, 0, 1)  # [T,Z,BL] -> [BL,T,Z]
            for ci in range(NCORES)
        ]
        outs.append(np.ascontiguousarray(np.concatenate(parts, 0), np.float32))
    return tuple(outs)
